# revision 17
# baseline (speedup 1.0000x reference)
"""DEDICOM decoder forward on 8 Trainium2 NeuronCores.

Math per relation k (k=0..7):
    M_k = diag(d_k) @ G @ diag(d_k)                  (64x64, host-precomputed)
    out[k, n] = sigmoid( (row_n @ M_k) . col_n )

v3 pipeline (data-parallel over N across 8 cores; per core 62500 samples
padded to 490 tiles x 128 consecutive samples):
  - row shipped to HBM as bf16 [SHARD_PAD, 128] (features zero-padded
    64->128 so the DMA xbar transpose is legal); per group one
    dma_start_transpose yields rowT [128, W*128] in SBUF (features on
    partitions 0:64) -- no PE transposes, no PSUM round trip.
  - col shipped as bf16 [SHARD_PAD, 64]; loaded sample-major
    (partition p = sample 128t+p).
  - PE: per tile T = rowT_tile.T @ M_all -> PSUM f32 [128, 512],
    4 tiles per PSUM batch (4 banks), double buffered (8 banks).
  - ACT: batched copy PSUM f32 -> SBUF bf16.
  - DVE: U = T * broadcast(col) (2x mode), fold 64->32->16.
  - GPSIMD: folds 16->8->4->2->1 (f32 out).
  - ACT: sigmoid per group; DMA out [SHARD_PAD, 8] f32.
"""

import sys

sys.path.insert(0, "/opt/trn_rl_repo")

import numpy as np
import ml_dtypes

import concourse.bass as bass
import concourse.bacc as bacc
import concourse.mybir as mybir
from concourse import tile
from concourse.bass_utils import run_bass_kernel_spmd

N, D, R = 500000, 64, 8
NCORES = 8
SHARD = N // NCORES            # 62500
NTILES = 490                   # tiles of 128 consecutive samples
SHARD_PAD = NTILES * 128       # 62720
W = 70                         # tiles per group; 7 groups
NGROUPS = NTILES // W
B = 4                          # tiles per PSUM batch
BF16 = mybir.dt.bfloat16
F32 = mybir.dt.float32

_CACHE: dict = {}


def _build_program():
    if "nc" in _CACHE:
        return _CACHE["nc"]

    import concourse.hw_specs as hw_specs

    hw_specs.TRN2Spec.GPSIMD_IMPL_EFFICIENCY["Add"] = 0.21

    nc = bacc.Bacc(
        "TRN2", target_bir_lowering=False, debug=False, num_devices=NCORES
    )

    row_d = nc.dram_tensor("row", [SHARD_PAD, 2 * D], BF16, kind="ExternalInput")
    col_d = nc.dram_tensor("col", [SHARD_PAD, D], BF16, kind="ExternalInput")
    mall_d = nc.dram_tensor("mall", [D, R * D], BF16, kind="ExternalInput")
    out_d = nc.dram_tensor("out", [SHARD_PAD, R], F32, kind="ExternalOutput")

    row_v = row_d.ap()                                     # [SHARD_PAD, 128]
    col_v = col_d.ap().rearrange("(t p) d -> p t d", p=128)  # [128, 490, 64]
    out_v = out_d.ap().rearrange("(t p) k -> p t k", p=128)  # [128, 490, 8]

    ADD = mybir.AluOpType.add
    MULT = mybir.AluOpType.mult
    GS = W * 128                                            # samples per group

    with tile.TileContext(nc) as tc:
        with (
            tc.tile_pool(name="const", bufs=1) as cpool,
            tc.tile_pool(name="rowt", bufs=3) as rpool,
            tc.tile_pool(name="io", bufs=3) as iopool,
            tc.tile_pool(name="psum_t", bufs=2, space="PSUM") as pt_pool,
            tc.tile_pool(name="tsb", bufs=2) as tpool,
            tc.tile_pool(name="u", bufs=2) as upool,
            tc.tile_pool(name="fold", bufs=2) as fpool,
            tc.tile_pool(name="out", bufs=2) as opool,
        ):
            mall = cpool.tile([D, R * D], BF16, tag="mall")
            nc.sync.dma_start(mall[:], mall_d.ap())

            def load_group(g):
                t0 = g * W
                rowT = rpool.tile([128, GS], BF16, tag="rowT")
                nc.sync.dma_start_transpose(
                    rowT[:], row_v[t0 * 128 : t0 * 128 + GS, :]
                )
                col_g = iopool.tile([128, W, D], BF16, tag="col_g")
                nc.sync.dma_start(col_g[:], col_v[:, t0 : t0 + W, :])
                return rowT, col_g

            def emit_folds(U3):
                U4 = fpool.tile([128, W, R, 8], BF16, tag="U4")
                nc.gpsimd.tensor_tensor(
                    out=U4[:], in0=U3[:, :, :, 0:8], in1=U3[:, :, :, 8:16], op=ADD
                )
                U5 = fpool.tile([128, W, R, 4], BF16, tag="U5")
                nc.gpsimd.tensor_tensor(
                    out=U5[:], in0=U4[:, :, :, 0:4], in1=U4[:, :, :, 4:8], op=ADD
                )
                U6 = fpool.tile([128, W, R, 2], BF16, tag="U6")
                nc.gpsimd.tensor_tensor(
                    out=U6[:], in0=U5[:, :, :, 0:2], in1=U5[:, :, :, 2:4], op=ADD
                )
                rec = opool.tile([128, W, R], F32, tag="rec")
                nc.gpsimd.tensor_tensor(
                    out=rec[:].unsqueeze(3),
                    in0=U6[:, :, :, 0:1],
                    in1=U6[:, :, :, 1:2],
                    op=ADD,
                )
                return rec

            def emit_out(rec, tg0):
                sig = opool.tile([128, W, R], F32, tag="sig")
                nc.scalar.activation(
                    sig[:], rec[:], mybir.ActivationFunctionType.Sigmoid
                )
                nc.scalar.dma_start(out_v[:, tg0 : tg0 + W, :], sig[:])

            loaded = load_group(0)
            pending = None  # (U3, t0) of the previous group, tail not yet done
            for g in range(NGROUPS):
                t0 = g * W
                rowT, col_g = loaded

                # the previous group's gpsimd fold chain runs concurrently
                # with this group's batch pipeline
                if pending is not None:
                    pend_rec = emit_folds(pending[0])

                U3 = fpool.tile([128, W, R, 16], BF16, tag="U3")

                for q0 in range(0, W, B):
                    bw = min(B, W - q0)
                    T_ps = pt_pool.tile([128, B, R * D], F32, tag="T")
                    for i in range(bw):
                        t = q0 + i
                        nc.tensor.matmul(
                            T_ps[:, i, :],
                            rowT[0:64, (t * 128) : (t * 128 + 128)],
                            mall[:],
                        )
                    T_sb = tpool.tile([128, B, R, D], BF16, tag="T_sb")
                    nc.scalar.copy(
                        T_sb[:, :bw, :, :].rearrange("p b k j -> p b (k j)"),
                        T_ps[:, :bw, :],
                    )
                    U = upool.tile([128, B, R, D], BF16, tag="U")
                    colb = (
                        col_g[:, q0 : q0 + bw, :]
                        .unsqueeze(2)
                        .broadcast_to([128, bw, R, D])
                    )
                    nc.vector.tensor_tensor(
                        out=U[:, :bw], in0=T_sb[:, :bw], in1=colb, op=MULT
                    )
                    U2 = upool.tile([128, B, R, 32], BF16, tag="U2")
                    nc.vector.tensor_tensor(
                        out=U2[:, :bw],
                        in0=U[:, :bw, :, 0:32],
                        in1=U[:, :bw, :, 32:64],
                        op=ADD,
                    )
                    nc.vector.tensor_tensor(
                        out=U3[:, q0 : q0 + bw],
                        in0=U2[:, :bw, :, 0:16],
                        in1=U2[:, :bw, :, 16:32],
                        op=ADD,
                    )

                if g + 1 < NGROUPS:
                    loaded = load_group(g + 1)

                # previous group's sigmoid + store: its fold chain has had a
                # whole group of runway, so the ACT queue won't stall on it
                if pending is not None:
                    emit_out(pend_rec, pending[1])
                pending = (U3, t0)

            # drain the last group's tail
            pend_rec = emit_folds(pending[0])
            emit_out(pend_rec, pending[1])

    nc.compile()
    _CACHE["nc"] = nc
    return nc


def _prep_inputs(inputs_row, inputs_col, global_interaction, local_variation):
    d = np.asarray(local_variation, np.float32)
    g = np.asarray(global_interaction, np.float32)
    # mall[i, (k, j)] = d[k, i] * G[i, j] * d[k, j]
    mall = np.einsum("ki,ij,kj->ikj", d, g, d).reshape(D, R * D)
    mall = np.ascontiguousarray(mall).astype(ml_dtypes.bfloat16)

    row16 = np.zeros((N, 2 * D), dtype=ml_dtypes.bfloat16)
    row16[:, :D] = np.asarray(inputs_row, np.float32)
    col16 = np.asarray(inputs_col, np.float32).astype(ml_dtypes.bfloat16)

    pad = SHARD_PAD - SHARD
    in_maps = []
    for c in range(NCORES):
        sl = slice(c * SHARD, (c + 1) * SHARD)
        rr = np.concatenate(
            [row16[sl], np.zeros((pad, 2 * D), ml_dtypes.bfloat16)]
        )
        cc = np.concatenate(
            [col16[sl], np.zeros((pad, D), ml_dtypes.bfloat16)]
        )
        in_maps.append(
            {
                "row": np.ascontiguousarray(rr),
                "col": np.ascontiguousarray(cc),
                "mall": mall,
            }
        )
    return in_maps


def kernel(inputs_row, inputs_col, global_interaction, local_variation):
    nc = _build_program()
    in_maps = _prep_inputs(
        inputs_row, inputs_col, global_interaction, local_variation
    )
    res = run_bass_kernel_spmd(nc, in_maps, list(range(NCORES)))
    outs = [res.results[c]["out"][:SHARD] for c in range(NCORES)]
    full = np.concatenate(outs, axis=0)  # [N, 8] f32
    return np.ascontiguousarray(full.T)  # [8, N]


if __name__ == "__main__":
    rng = np.random.default_rng(0)
    inputs = {
        "inputs_row": rng.standard_normal((N, D), dtype=np.float32),
        "inputs_col": rng.standard_normal((N, D), dtype=np.float32),
        "global_interaction": rng.uniform(-0.2, 0.2, (D, D)).astype(np.float32),
        "local_variation": rng.uniform(-0.3, 0.3, (R, D)).astype(np.float32),
    }
    out = kernel(**inputs)
    print("out", out.shape, out.dtype, out[:, :3])


# revision 18
# speedup vs baseline: 1.2592x; 1.2592x over previous
"""DEDICOM decoder forward on 8 Trainium2 NeuronCores.

Math per relation k (k=0..7):
    M_k = diag(d_k) @ G @ diag(d_k)                  (64x64, host-precomputed)
    out[k, n] = sigmoid( (row_n @ M_k) . col_n )

v3 pipeline (data-parallel over N across 8 cores; per core 62500 samples
padded to 490 tiles x 128 consecutive samples):
  - row shipped to HBM as bf16 [SHARD_PAD, 128] (features zero-padded
    64->128 so the DMA xbar transpose is legal); per group one
    dma_start_transpose yields rowT [128, W*128] in SBUF (features on
    partitions 0:64) -- no PE transposes, no PSUM round trip.
  - col shipped as bf16 [SHARD_PAD, 64]; loaded sample-major
    (partition p = sample 128t+p).
  - PE: per tile T = rowT_tile.T @ M_all -> PSUM f32 [128, 512],
    4 tiles per PSUM batch (4 banks), double buffered (8 banks).
  - ACT: batched copy PSUM f32 -> SBUF bf16.
  - DVE: U = T * broadcast(col) (2x mode), folds 64->32->16 per batch;
    per-group tail folds 16->8->4->2->1 (f32 out), software-pipelined
    into the next group so the in-order engine queues never stall.
  - ACT: sigmoid per group; DMA out [SHARD_PAD, 8] f32 via scalar HWDGE.
"""

import sys

sys.path.insert(0, "/opt/trn_rl_repo")

import numpy as np
import ml_dtypes

import concourse.bass as bass
import concourse.bacc as bacc
import concourse.mybir as mybir
from concourse import tile
from concourse.bass_utils import run_bass_kernel_spmd

N, D, R = 500000, 64, 8
NCORES = 8
SHARD = N // NCORES            # 62500
NTILES = 490                   # tiles of 128 consecutive samples
SHARD_PAD = NTILES * 128       # 62720
W = 70                         # tiles per group; 7 groups
NGROUPS = NTILES // W
B = 4                          # tiles per PSUM batch
BF16 = mybir.dt.bfloat16
F32 = mybir.dt.float32

_CACHE: dict = {}


def _build_program():
    if "nc" in _CACHE:
        return _CACHE["nc"]

    nc = bacc.Bacc(
        "TRN2", target_bir_lowering=False, debug=False, num_devices=NCORES
    )

    row_d = nc.dram_tensor("row", [SHARD_PAD, 2 * D], BF16, kind="ExternalInput")
    col_d = nc.dram_tensor("col", [SHARD_PAD, D], BF16, kind="ExternalInput")
    mall_d = nc.dram_tensor("mall", [D, R * D], BF16, kind="ExternalInput")
    out_d = nc.dram_tensor("out", [SHARD_PAD, R], F32, kind="ExternalOutput")

    row_v = row_d.ap()                                     # [SHARD_PAD, 128]
    col_v = col_d.ap().rearrange("(t p) d -> p t d", p=128)  # [128, 490, 64]
    out_v = out_d.ap().rearrange("(t p) k -> p t k", p=128)  # [128, 490, 8]

    ADD = mybir.AluOpType.add
    MULT = mybir.AluOpType.mult
    GS = W * 128                                            # samples per group

    with tile.TileContext(nc) as tc:
        with (
            tc.tile_pool(name="const", bufs=1) as cpool,
            tc.tile_pool(name="rowt", bufs=3) as rpool,
            tc.tile_pool(name="io", bufs=3) as iopool,
            tc.tile_pool(name="psum_t", bufs=2, space="PSUM") as pt_pool,
            tc.tile_pool(name="tsb", bufs=2) as tpool,
            tc.tile_pool(name="u", bufs=2) as upool,
            tc.tile_pool(name="fold", bufs=2) as fpool,
            tc.tile_pool(name="out", bufs=2) as opool,
        ):
            mall = cpool.tile([D, R * D], BF16, tag="mall")
            nc.sync.dma_start(mall[:], mall_d.ap())

            def load_group(g):
                t0 = g * W
                rowT = rpool.tile([128, GS], BF16, tag="rowT")
                nc.sync.dma_start_transpose(
                    rowT[:], row_v[t0 * 128 : t0 * 128 + GS, :]
                )
                col_g = iopool.tile([128, W, D], BF16, tag="col_g")
                nc.sync.dma_start(col_g[:], col_v[:, t0 : t0 + W, :])
                return rowT, col_g

            def emit_folds(U3):
                U4 = fpool.tile([128, W, R, 8], BF16, tag="U4")
                nc.vector.tensor_tensor(
                    out=U4[:], in0=U3[:, :, :, 0:8], in1=U3[:, :, :, 8:16], op=ADD
                )
                U5 = fpool.tile([128, W, R, 4], BF16, tag="U5")
                nc.vector.tensor_tensor(
                    out=U5[:], in0=U4[:, :, :, 0:4], in1=U4[:, :, :, 4:8], op=ADD
                )
                U6 = fpool.tile([128, W, R, 2], BF16, tag="U6")
                nc.vector.tensor_tensor(
                    out=U6[:], in0=U5[:, :, :, 0:2], in1=U5[:, :, :, 2:4], op=ADD
                )
                rec = opool.tile([128, W, R], F32, tag="rec")
                nc.vector.tensor_tensor(
                    out=rec[:].unsqueeze(3),
                    in0=U6[:, :, :, 0:1],
                    in1=U6[:, :, :, 1:2],
                    op=ADD,
                )
                return rec

            def emit_out(rec, tg0):
                sig = opool.tile([128, W, R], F32, tag="sig")
                nc.scalar.activation(
                    sig[:], rec[:], mybir.ActivationFunctionType.Sigmoid
                )
                nc.scalar.dma_start(out_v[:, tg0 : tg0 + W, :], sig[:])

            loaded = load_group(0)
            pending = None  # (U3, t0) of the previous group, tail not yet done
            for g in range(NGROUPS):
                t0 = g * W
                rowT, col_g = loaded

                # the previous group's fold tail runs concurrently with
                # this group's batch pipeline
                if pending is not None:
                    pend_rec = emit_folds(pending[0])

                U3 = fpool.tile([128, W, R, 16], BF16, tag="U3")

                for q0 in range(0, W, B):
                    bw = min(B, W - q0)
                    T_ps = pt_pool.tile([128, B, R * D], F32, tag="T")
                    for i in range(bw):
                        t = q0 + i
                        nc.tensor.matmul(
                            T_ps[:, i, :],
                            rowT[0:64, (t * 128) : (t * 128 + 128)],
                            mall[:],
                        )
                    T_sb = tpool.tile([128, B, R, D], BF16, tag="T_sb")
                    nc.scalar.copy(
                        T_sb[:, :bw, :, :].rearrange("p b k j -> p b (k j)"),
                        T_ps[:, :bw, :],
                    )
                    U = upool.tile([128, B, R, D], BF16, tag="U")
                    colb = (
                        col_g[:, q0 : q0 + bw, :]
                        .unsqueeze(2)
                        .broadcast_to([128, bw, R, D])
                    )
                    nc.vector.tensor_tensor(
                        out=U[:, :bw], in0=T_sb[:, :bw], in1=colb, op=MULT
                    )
                    U2 = upool.tile([128, B, R, 32], BF16, tag="U2")
                    nc.vector.tensor_tensor(
                        out=U2[:, :bw],
                        in0=U[:, :bw, :, 0:32],
                        in1=U[:, :bw, :, 32:64],
                        op=ADD,
                    )
                    nc.vector.tensor_tensor(
                        out=U3[:, q0 : q0 + bw],
                        in0=U2[:, :bw, :, 0:16],
                        in1=U2[:, :bw, :, 16:32],
                        op=ADD,
                    )

                if g + 1 < NGROUPS:
                    loaded = load_group(g + 1)

                # previous group's sigmoid + store: its fold chain has had a
                # whole group of runway, so the ACT queue won't stall on it
                if pending is not None:
                    emit_out(pend_rec, pending[1])
                pending = (U3, t0)

            # drain the last group's tail
            pend_rec = emit_folds(pending[0])
            emit_out(pend_rec, pending[1])

    nc.compile()
    _CACHE["nc"] = nc
    return nc


def _prep_inputs(inputs_row, inputs_col, global_interaction, local_variation):
    d = np.asarray(local_variation, np.float32)
    g = np.asarray(global_interaction, np.float32)
    # mall[i, (k, j)] = d[k, i] * G[i, j] * d[k, j]
    mall = np.einsum("ki,ij,kj->ikj", d, g, d).reshape(D, R * D)
    mall = np.ascontiguousarray(mall).astype(ml_dtypes.bfloat16)

    row16 = np.zeros((N, 2 * D), dtype=ml_dtypes.bfloat16)
    row16[:, :D] = np.asarray(inputs_row, np.float32)
    col16 = np.asarray(inputs_col, np.float32).astype(ml_dtypes.bfloat16)

    pad = SHARD_PAD - SHARD
    in_maps = []
    for c in range(NCORES):
        sl = slice(c * SHARD, (c + 1) * SHARD)
        rr = np.concatenate(
            [row16[sl], np.zeros((pad, 2 * D), ml_dtypes.bfloat16)]
        )
        cc = np.concatenate(
            [col16[sl], np.zeros((pad, D), ml_dtypes.bfloat16)]
        )
        in_maps.append(
            {
                "row": np.ascontiguousarray(rr),
                "col": np.ascontiguousarray(cc),
                "mall": mall,
            }
        )
    return in_maps


def kernel(inputs_row, inputs_col, global_interaction, local_variation):
    nc = _build_program()
    in_maps = _prep_inputs(
        inputs_row, inputs_col, global_interaction, local_variation
    )
    res = run_bass_kernel_spmd(nc, in_maps, list(range(NCORES)))
    outs = [res.results[c]["out"][:SHARD] for c in range(NCORES)]
    full = np.concatenate(outs, axis=0)  # [N, 8] f32
    return np.ascontiguousarray(full.T)  # [8, N]


if __name__ == "__main__":
    rng = np.random.default_rng(0)
    inputs = {
        "inputs_row": rng.standard_normal((N, D), dtype=np.float32),
        "inputs_col": rng.standard_normal((N, D), dtype=np.float32),
        "global_interaction": rng.uniform(-0.2, 0.2, (D, D)).astype(np.float32),
        "local_variation": rng.uniform(-0.3, 0.3, (R, D)).astype(np.float32),
    }
    out = kernel(**inputs)
    print("out", out.shape, out.dtype, out[:, :3])


# revision 19
# speedup vs baseline: 1.2758x; 1.0132x over previous
"""DEDICOM decoder forward on 8 Trainium2 NeuronCores.

Math per relation k (k=0..7):
    M_k = diag(d_k) @ G @ diag(d_k)                  (64x64, host-precomputed)
    out[k, n] = sigmoid( (row_n @ M_k) . col_n )

v3 pipeline (data-parallel over N across 8 cores; per core 62500 samples
padded to 490 tiles x 128 consecutive samples):
  - row shipped to HBM as bf16 [SHARD_PAD, 128] (features zero-padded
    64->128 so the DMA xbar transpose is legal); per group one
    dma_start_transpose yields rowT [128, W*128] in SBUF (features on
    partitions 0:64) -- no PE transposes, no PSUM round trip.
  - col shipped as bf16 [SHARD_PAD, 64]; loaded sample-major
    (partition p = sample 128t+p).
  - PE: per tile T = rowT_tile.T @ M_all -> PSUM f32 [128, 512],
    4 tiles per PSUM batch (4 banks), double buffered (8 banks).
  - ACT: batched copy PSUM f32 -> SBUF bf16.
  - DVE: U = T * broadcast(col) (2x mode), folds 64->32->16 per batch;
    per-group tail folds 16->8->4->2->1 (f32 out), software-pipelined
    into the next group so the in-order engine queues never stall.
  - ACT: sigmoid per group; DMA out [SHARD_PAD, 8] f32 via scalar HWDGE.
"""

import sys

sys.path.insert(0, "/opt/trn_rl_repo")

import numpy as np
import ml_dtypes

import concourse.bass as bass
import concourse.bacc as bacc
import concourse.mybir as mybir
from concourse import tile
from concourse.bass_utils import run_bass_kernel_spmd

N, D, R = 500000, 64, 8
NCORES = 8
SHARD = N // NCORES            # 62500
NTILES = 490                   # tiles of 128 consecutive samples
SHARD_PAD = NTILES * 128       # 62720
W = 70                         # tiles per group; 7 groups
NGROUPS = NTILES // W
B = 4                          # tiles per PSUM batch
BF16 = mybir.dt.bfloat16
F32 = mybir.dt.float32

_CACHE: dict = {}


def _build_program():
    if "nc" in _CACHE:
        return _CACHE["nc"]

    nc = bacc.Bacc(
        "TRN2", target_bir_lowering=False, debug=False, num_devices=NCORES
    )

    row_d = nc.dram_tensor("row", [SHARD_PAD, 2 * D], BF16, kind="ExternalInput")
    col_d = nc.dram_tensor("col", [SHARD_PAD, D], BF16, kind="ExternalInput")
    mall_d = nc.dram_tensor("mall", [D, R * D], BF16, kind="ExternalInput")
    out_d = nc.dram_tensor("out", [SHARD_PAD, R], F32, kind="ExternalOutput")

    row_v = row_d.ap()                                     # [SHARD_PAD, 128]
    col_v = col_d.ap().rearrange("(t p) d -> p t d", p=128)  # [128, 490, 64]
    out_v = out_d.ap().rearrange("(t p) k -> p t k", p=128)  # [128, 490, 8]

    ADD = mybir.AluOpType.add
    MULT = mybir.AluOpType.mult
    GS = W * 128                                            # samples per group

    with tile.TileContext(nc) as tc:
        with (
            tc.tile_pool(name="const", bufs=1) as cpool,
            tc.tile_pool(name="rowt", bufs=3) as rpool,
            tc.tile_pool(name="io", bufs=3) as iopool,
            tc.tile_pool(name="psum_t", bufs=2, space="PSUM") as pt_pool,
            tc.tile_pool(name="tsb", bufs=3) as tpool,
            tc.tile_pool(name="u", bufs=3) as upool,
            tc.tile_pool(name="fold", bufs=2) as fpool,
            tc.tile_pool(name="out", bufs=2) as opool,
        ):
            mall = cpool.tile([D, R * D], BF16, tag="mall")
            nc.sync.dma_start(mall[:], mall_d.ap())

            def load_group(g):
                t0 = g * W
                rowT = rpool.tile([128, GS], BF16, tag="rowT")
                nc.sync.dma_start_transpose(
                    rowT[:], row_v[t0 * 128 : t0 * 128 + GS, :]
                )
                col_g = iopool.tile([128, W, D], BF16, tag="col_g")
                nc.sync.dma_start(col_g[:], col_v[:, t0 : t0 + W, :])
                return rowT, col_g

            def emit_folds(U3):
                U4 = fpool.tile([128, W, R, 8], BF16, tag="U4")
                nc.vector.tensor_tensor(
                    out=U4[:], in0=U3[:, :, :, 0:8], in1=U3[:, :, :, 8:16], op=ADD
                )
                U5 = fpool.tile([128, W, R, 4], BF16, tag="U5")
                nc.vector.tensor_tensor(
                    out=U5[:], in0=U4[:, :, :, 0:4], in1=U4[:, :, :, 4:8], op=ADD
                )
                U6 = fpool.tile([128, W, R, 2], BF16, tag="U6")
                nc.vector.tensor_tensor(
                    out=U6[:], in0=U5[:, :, :, 0:2], in1=U5[:, :, :, 2:4], op=ADD
                )
                rec = opool.tile([128, W, R], F32, tag="rec")
                nc.vector.tensor_tensor(
                    out=rec[:].unsqueeze(3),
                    in0=U6[:, :, :, 0:1],
                    in1=U6[:, :, :, 1:2],
                    op=ADD,
                )
                return rec

            def emit_out(rec, tg0):
                sig = opool.tile([128, W, R], F32, tag="sig")
                nc.scalar.activation(
                    sig[:], rec[:], mybir.ActivationFunctionType.Sigmoid
                )
                nc.scalar.dma_start(out_v[:, tg0 : tg0 + W, :], sig[:])

            loaded = load_group(0)
            pending = None  # (U3, t0) of the previous group, tail not yet done
            for g in range(NGROUPS):
                t0 = g * W
                rowT, col_g = loaded

                # the previous group's fold tail runs concurrently with
                # this group's batch pipeline
                if pending is not None:
                    pend_rec = emit_folds(pending[0])

                U3 = fpool.tile([128, W, R, 16], BF16, tag="U3")

                for q0 in range(0, W, B):
                    bw = min(B, W - q0)
                    T_ps = pt_pool.tile([128, B, R * D], F32, tag="T")
                    for i in range(bw):
                        t = q0 + i
                        nc.tensor.matmul(
                            T_ps[:, i, :],
                            rowT[0:64, (t * 128) : (t * 128 + 128)],
                            mall[:],
                        )
                    T_sb = tpool.tile([128, B, R, D], BF16, tag="T_sb")
                    nc.scalar.copy(
                        T_sb[:, :bw, :, :].rearrange("p b k j -> p b (k j)"),
                        T_ps[:, :bw, :],
                    )
                    U = upool.tile([128, B, R, D], BF16, tag="U")
                    colb = (
                        col_g[:, q0 : q0 + bw, :]
                        .unsqueeze(2)
                        .broadcast_to([128, bw, R, D])
                    )
                    nc.vector.tensor_tensor(
                        out=U[:, :bw], in0=T_sb[:, :bw], in1=colb, op=MULT
                    )
                    U2 = upool.tile([128, B, R, 32], BF16, tag="U2")
                    nc.vector.tensor_tensor(
                        out=U2[:, :bw],
                        in0=U[:, :bw, :, 0:32],
                        in1=U[:, :bw, :, 32:64],
                        op=ADD,
                    )
                    nc.vector.tensor_tensor(
                        out=U3[:, q0 : q0 + bw],
                        in0=U2[:, :bw, :, 0:16],
                        in1=U2[:, :bw, :, 16:32],
                        op=ADD,
                    )

                if g + 1 < NGROUPS:
                    loaded = load_group(g + 1)

                # previous group's sigmoid + store: its fold chain has had a
                # whole group of runway, so the ACT queue won't stall on it
                if pending is not None:
                    emit_out(pend_rec, pending[1])
                pending = (U3, t0)

            # drain the last group's tail
            pend_rec = emit_folds(pending[0])
            emit_out(pend_rec, pending[1])

    nc.compile()
    _CACHE["nc"] = nc
    return nc


def _prep_inputs(inputs_row, inputs_col, global_interaction, local_variation):
    d = np.asarray(local_variation, np.float32)
    g = np.asarray(global_interaction, np.float32)
    # mall[i, (k, j)] = d[k, i] * G[i, j] * d[k, j]
    mall = np.einsum("ki,ij,kj->ikj", d, g, d).reshape(D, R * D)
    mall = np.ascontiguousarray(mall).astype(ml_dtypes.bfloat16)

    row16 = np.zeros((N, 2 * D), dtype=ml_dtypes.bfloat16)
    row16[:, :D] = np.asarray(inputs_row, np.float32)
    col16 = np.asarray(inputs_col, np.float32).astype(ml_dtypes.bfloat16)

    pad = SHARD_PAD - SHARD
    in_maps = []
    for c in range(NCORES):
        sl = slice(c * SHARD, (c + 1) * SHARD)
        rr = np.concatenate(
            [row16[sl], np.zeros((pad, 2 * D), ml_dtypes.bfloat16)]
        )
        cc = np.concatenate(
            [col16[sl], np.zeros((pad, D), ml_dtypes.bfloat16)]
        )
        in_maps.append(
            {
                "row": np.ascontiguousarray(rr),
                "col": np.ascontiguousarray(cc),
                "mall": mall,
            }
        )
    return in_maps


def kernel(inputs_row, inputs_col, global_interaction, local_variation):
    nc = _build_program()
    in_maps = _prep_inputs(
        inputs_row, inputs_col, global_interaction, local_variation
    )
    res = run_bass_kernel_spmd(nc, in_maps, list(range(NCORES)))
    outs = [res.results[c]["out"][:SHARD] for c in range(NCORES)]
    full = np.concatenate(outs, axis=0)  # [N, 8] f32
    return np.ascontiguousarray(full.T)  # [8, N]


if __name__ == "__main__":
    rng = np.random.default_rng(0)
    inputs = {
        "inputs_row": rng.standard_normal((N, D), dtype=np.float32),
        "inputs_col": rng.standard_normal((N, D), dtype=np.float32),
        "global_interaction": rng.uniform(-0.2, 0.2, (D, D)).astype(np.float32),
        "local_variation": rng.uniform(-0.3, 0.3, (R, D)).astype(np.float32),
    }
    out = kernel(**inputs)
    print("out", out.shape, out.dtype, out[:, :3])
